# revision 4
# baseline (speedup 1.0000x reference)
"""Trainium2 Bass kernel for an attention-style graph convolution (GAT layer).

v7 = v3 (4x2 sharding, ACT relu-split offload) with progressive tiling:
the first j-chunks are processed as small units (1,1,2 chunks) so the
DMA->ts->tt->matmul pipeline fills fast, then steady-state runs on 4-chunk
units (low per-op overhead). Drain uses vector-only copies into one batched
tile and 4 grouped output DMAs on both rings.

Math (mask stream pre-scaled: mt[j,i] = m[i,j]*es2a_j, fp16):
    n[j,i] = mt * max(es1b_i*es2m_j, 1) = mt * (1 + relu(es1b_i*es2m_j - 1))
DVE-units: u = max(es1b*es2m_j, 1) (ts) ; n = u*mt (tt) ; 16 mm/chunk
ACT-units: w = relu(es1b*es2m_j - 1) (ACT engine) ; n1 = w*mt (tt)
           32 mm/chunk (n1.T@g2 plus mt.T@g2 -- the "+1" chain on the PE)
j-half partials summed across core pairs on the host, then divide + elu.
"""

import ml_dtypes
import numpy as np

import concourse.bacc as bacc
import concourse.bass as bass
import concourse.mybir as mybir
import concourse.tile as tile
from concourse import bass_utils

F32 = mybir.dt.float32
FP16 = mybir.dt.float16
AF = mybir.ActivationFunctionType
OP = mybir.AluOpType

N = 8192
K = 256
F = 128
ALPHA = 0.2
NCORES = 8
MI = 2048         # i-columns per core
MJ = 4096         # j-rows per core
P = 128
NJ = MJ // P      # 32 j-chunks
NIT = MI // P     # 16 i-blocks

UNIT_SIZES = [1, 1, 2, 4, 4, 4, 4, 4, 4, 4]   # chunks per unit (sum 32)
ACT_UNITS = frozenset({4, 6, 8})              # units built on the ACT engine
LAG = 2                                        # pipeline depth in units


def _broadcast_ap(row_ap, nparts):
    return bass.AP(
        tensor=row_ap.tensor,
        offset=row_ap.offset,
        ap=[[0, nparts]] + [list(d) for d in row_ap.ap],
    )


def build_program():
    nc = bacc.Bacc("TRN2", target_bir_lowering=False)

    mt_d = nc.dram_tensor("mt", (MJ, MI), FP16, kind="ExternalInput")
    g2_d = nc.dram_tensor("g2", (P, NJ * (F + 1)), FP16, kind="ExternalInput")
    es1b_d = nc.dram_tensor("es1b", (1, MI), FP16, kind="ExternalInput")
    es2m_d = nc.dram_tensor("es2m", (P, NJ), F32, kind="ExternalInput")
    out_d = nc.dram_tensor("out", (MI, F + 1), F32, kind="ExternalOutput")

    with tile.TileContext(nc) as tc:
        with (
            tc.tile_pool(name="consts", bufs=1) as consts,
            tc.tile_pool(name="adjp", bufs=4) as adjp,
            tc.tile_pool(name="up", bufs=2) as up,
            tc.tile_pool(name="ntp", bufs=3) as ntp,
            tc.tile_pool(name="gsp", bufs=2) as gsp,
            tc.tile_pool(name="outp", bufs=1) as outp,
            tc.tile_pool(name="ps_acc", bufs=1, space="PSUM") as ps_acc,
        ):
            es2m = consts.tile([P, NJ], F32, tag="es2m")
            es1b = consts.tile([P, MI], FP16, tag="es1b")
            neg1 = consts.tile([P, 1], F32, tag="neg1")
            nc.gpsimd.memset(neg1[:], -1.0)
            # es-vectors on sync; the first mask units go on scalar so the
            # latency-critical chunk-0 transfer is not queued behind es1b
            nc.sync.dma_start(out=es2m[:], in_=es2m_d[:, :])
            nc.sync.dma_start(out=es1b[:], in_=_broadcast_ap(es1b_d[:, :], P))

            accs = [
                ps_acc.tile([P, 512], F32, tag=f"acc{b}", name=f"acc{b}")
                for b in range(8)
            ]

            def acc_slice(it):
                return accs[it // 2][:, (it % 2) * 256 : (it % 2) * 256 + F + 1]

            mt_r = mt_d.rearrange("(c p) m -> p c m", p=P)
            unit_off = np.cumsum([0] + UNIT_SIZES).tolist()

            pend = []
            gs_slab = [None]

            def phase_a(un):
                off, sz = unit_off[un], UNIT_SIZES[un]
                if off % 8 == 0:
                    g8 = off // 8
                    gs = gsp.tile([P, 8 * (F + 1)], FP16, tag="gs")
                    nc.sync.dma_start(
                        out=gs[:],
                        in_=g2_d[:, g8 * 8 * (F + 1) : (g8 + 1) * 8 * (F + 1)],
                    )
                    gs_slab[0] = gs
                adj_t = adjp.tile([P, 4, MI], FP16, tag="adj")
                eng = nc.scalar if un % 2 == 0 else nc.sync
                eng.dma_start(
                    out=adj_t[:, :sz, :], in_=mt_r[:, off : off + sz, :]
                )
                pend.append((un, adj_t, gs_slab[0]))

            def phase_c():
                un, adj_t, gs = pend.pop(0)
                off, sz = unit_off[un], UNIT_SIZES[un]
                is_act = un in ACT_UNITS
                u_t = up.tile([P, 4, MI], FP16, tag="u_t")
                for q in range(sz):
                    jc = off + q
                    if is_act:
                        nc.scalar.activation(
                            u_t[:, q, :], es1b[:], AF.Relu,
                            bias=neg1[:], scale=es2m[:, jc : jc + 1],
                        )
                    else:
                        nc.vector.tensor_scalar(
                            out=u_t[:, q, :],
                            in0=es1b[:],
                            scalar1=es2m[:, jc : jc + 1],
                            scalar2=1.0,
                            op0=OP.mult,
                            op1=OP.max,
                        )
                n_t = ntp.tile([P, 4, MI], FP16, tag="n_t")
                nc.vector.tensor_tensor(
                    out=n_t[:, :sz, :], in0=u_t[:, :sz, :],
                    in1=adj_t[:, :sz, :], op=OP.mult,
                )
                for q in range(sz):
                    jc = off + q
                    gsl = gs[:, (jc % 8) * (F + 1) : (jc % 8) * (F + 1) + F + 1]
                    for it in range(NIT):
                        nc.tensor.matmul(
                            acc_slice(it),
                            n_t[:, q, it * P : (it + 1) * P],
                            gsl,
                            start=(jc == 0 and it % 2 == 0),
                            stop=(jc == NJ - 1),
                            skip_group_check=True,
                        )
                        if is_act:
                            nc.tensor.matmul(
                                acc_slice(it),
                                adj_t[:, q, it * P : (it + 1) * P],
                                gsl,
                                start=False,
                                stop=False,
                                skip_group_check=True,
                            )

            nunits = len(UNIT_SIZES)
            for un in range(nunits):
                phase_a(un)
                if un >= LAG:
                    phase_c()
            while pend:
                phase_c()

            # drain: vector-only copies into one batched tile, grouped DMAs
            out_r = out_d.rearrange("(c p) f -> p c f", p=P)
            res = outp.tile([P, NIT, F + 1], F32, tag="res")
            for g in range(4):
                for k in range(4):
                    it = 4 * g + k
                    nc.vector.tensor_copy(res[:, it, :], acc_slice(it))
                eng = nc.sync if g % 2 == 0 else nc.scalar
                eng.dma_start(
                    out=out_r[:, 4 * g : 4 * g + 4, :],
                    in_=res[:, 4 * g : 4 * g + 4, :],
                )

    nc.compile()
    return nc


_NC_CACHE = [None]


def _get_nc():
    if _NC_CACHE[0] is None:
        _NC_CACHE[0] = build_program()
    return _NC_CACHE[0]


def host_prepare(x, adj, W, a):
    h64 = x.astype(np.float64) @ W.astype(np.float64)
    s1 = h64 @ a[:F, 0].astype(np.float64)
    s2 = h64 @ a[F:, 0].astype(np.float64)
    es2a = np.exp(ALPHA * s2)
    es2m = np.exp((1.0 - ALPHA) * s2)
    g2 = np.empty((N, F + 1), np.float64)
    g2[:, :F] = h64
    g2[:, F] = 1.0
    g2 = g2.astype(np.float16)
    es1b = np.exp((1.0 - ALPHA) * s1).astype(np.float16)
    maskT = adj.T > 0
    mt_full = np.where(maskT, es2a[:, None], 0.0).astype(np.float16)

    in_maps = []
    for c in range(NCORES):
        si = c % 4
        hj = c // 4
        isl = slice(si * MI, (si + 1) * MI)
        jsl = slice(hj * MJ, (hj + 1) * MJ)
        g2h = np.ascontiguousarray(
            g2[jsl].reshape(NJ, P, F + 1).transpose(1, 0, 2).reshape(P, NJ * (F + 1))
        )
        es2mh = np.ascontiguousarray(es2m[jsl].reshape(NJ, P).T.astype(np.float32))
        in_maps.append(
            {
                "mt": np.ascontiguousarray(mt_full[jsl, isl]),
                "g2": g2h,
                "es1b": es1b[isl].reshape(1, MI),
                "es2m": es2mh,
            }
        )
    return in_maps


def kernel(x, adj, W, a, _trace=False):
    x = np.asarray(x)
    adj = np.asarray(adj)
    W = np.asarray(W)
    a = np.asarray(a)

    in_maps = host_prepare(x, adj, W, a)
    nc = _get_nc()
    # warmup execution: brings device clocks/p-states up so the measured
    # run is not penalized by cold-start ramp
    bass_utils.run_bass_kernel_spmd(
        nc, in_maps, core_ids=list(range(NCORES)), trace=False
    )
    res = bass_utils.run_bass_kernel_spmd(
        nc, in_maps, core_ids=list(range(NCORES)), trace=_trace
    )
    slabs = []
    for si in range(4):
        slabs.append(res.results[si]["out"] + res.results[si + 4]["out"])
    nd = np.concatenate(slabs, axis=0)
    hp = nd[:, :F] / nd[:, F : F + 1]
    out = np.where(hp > 0, hp, np.expm1(np.minimum(hp, 0.0))).astype(np.float32)
    if _trace:
        return out, res
    return out


# revision 5
# speedup vs baseline: 1.0082x; 1.0082x over previous
"""Trainium2 Bass kernel for an attention-style graph convolution (GAT layer).

v7 = v3 (4x2 sharding, ACT relu-split offload) with progressive tiling:
the first j-chunks are processed as small units (1,1,2 chunks) so the
DMA->ts->tt->matmul pipeline fills fast, then steady-state runs on 4-chunk
units (low per-op overhead). Drain uses vector-only copies into one batched
tile and 4 grouped output DMAs on both rings.

Math (mask stream pre-scaled: mt[j,i] = m[i,j]*es2a_j, fp16):
    n[j,i] = mt * max(es1b_i*es2m_j, 1) = mt * (1 + relu(es1b_i*es2m_j - 1))
DVE-units: u = max(es1b*es2m_j, 1) (ts) ; n = u*mt (tt) ; 16 mm/chunk
ACT-units: w = relu(es1b*es2m_j - 1) (ACT engine) ; n1 = w*mt (tt)
           32 mm/chunk (n1.T@g2 plus mt.T@g2 -- the "+1" chain on the PE)
j-half partials summed across core pairs on the host, then divide + elu.
"""

import ml_dtypes
import numpy as np

import concourse.bacc as bacc
import concourse.bass as bass
import concourse.mybir as mybir
import concourse.tile as tile
from concourse import bass_utils

F32 = mybir.dt.float32
FP16 = mybir.dt.float16
AF = mybir.ActivationFunctionType
OP = mybir.AluOpType

N = 8192
K = 256
F = 128
ALPHA = 0.2
NCORES = 8
MI = 2048         # i-columns per core
MJ = 4096         # j-rows per core
P = 128
NJ = MJ // P      # 32 j-chunks
NIT = MI // P     # 16 i-blocks

UNIT_SIZES = [1, 1, 2, 4, 4, 4, 4, 4, 4, 4]   # chunks per unit (sum 32)
ACT_UNITS = frozenset({4, 6, 8})              # units built on the ACT engine
LAG = 2                                        # pipeline depth in units


def _broadcast_ap(row_ap, nparts):
    return bass.AP(
        tensor=row_ap.tensor,
        offset=row_ap.offset,
        ap=[[0, nparts]] + [list(d) for d in row_ap.ap],
    )


def build_program():
    nc = bacc.Bacc("TRN2", target_bir_lowering=False)

    mt_d = nc.dram_tensor("mt", (MJ, MI), FP16, kind="ExternalInput")
    g2_d = nc.dram_tensor("g2", (P, NJ * (F + 1)), FP16, kind="ExternalInput")
    es1b_d = nc.dram_tensor("es1b", (1, MI), FP16, kind="ExternalInput")
    es2m_d = nc.dram_tensor("es2m", (P, NJ), F32, kind="ExternalInput")
    out_d = nc.dram_tensor("out", (MI, F + 1), F32, kind="ExternalOutput")

    with tile.TileContext(nc) as tc:
        with (
            tc.tile_pool(name="consts", bufs=1) as consts,
            tc.tile_pool(name="adjp", bufs=4) as adjp,
            tc.tile_pool(name="up", bufs=2) as up,
            tc.tile_pool(name="ntp", bufs=3) as ntp,
            tc.tile_pool(name="gsp", bufs=2) as gsp,
            tc.tile_pool(name="outp", bufs=1) as outp,
            tc.tile_pool(name="ps_acc", bufs=1, space="PSUM") as ps_acc,
        ):
            es2m = consts.tile([P, NJ], F32, tag="es2m")
            es1b = consts.tile([P, MI], FP16, tag="es1b")
            neg1 = consts.tile([P, 1], F32, tag="neg1")
            nc.gpsimd.memset(neg1[:], -1.0)
            # es-vectors on sync; the first mask units go on scalar so the
            # latency-critical chunk-0 transfer is not queued behind es1b
            nc.sync.dma_start(out=es2m[:], in_=es2m_d[:, :])
            nc.sync.dma_start(out=es1b[:], in_=_broadcast_ap(es1b_d[:, :], P))

            accs = [
                ps_acc.tile([P, 512], F32, tag=f"acc{b}", name=f"acc{b}")
                for b in range(8)
            ]

            def acc_slice(it):
                return accs[it // 2][:, (it % 2) * 256 : (it % 2) * 256 + F + 1]

            mt_r = mt_d.rearrange("(c p) m -> p c m", p=P)
            unit_off = np.cumsum([0] + UNIT_SIZES).tolist()

            pend = []
            gs_slab = [None]

            def phase_a(un):
                off, sz = unit_off[un], UNIT_SIZES[un]
                if off % 8 == 0:
                    g8 = off // 8
                    gs = gsp.tile([P, 8 * (F + 1)], FP16, tag="gs")
                    nc.sync.dma_start(
                        out=gs[:],
                        in_=g2_d[:, g8 * 8 * (F + 1) : (g8 + 1) * 8 * (F + 1)],
                    )
                    gs_slab[0] = gs
                adj_t = adjp.tile([P, 4, MI], FP16, tag="adj")
                eng = nc.scalar if un % 2 == 0 else nc.sync
                eng.dma_start(
                    out=adj_t[:, :sz, :], in_=mt_r[:, off : off + sz, :]
                )
                pend.append((un, adj_t, gs_slab[0]))

            def phase_c():
                un, adj_t, gs = pend.pop(0)
                off, sz = unit_off[un], UNIT_SIZES[un]
                is_act = un in ACT_UNITS
                u_t = up.tile([P, 4, MI], FP16, tag="u_t")
                for q in range(sz):
                    jc = off + q
                    if is_act:
                        nc.scalar.activation(
                            u_t[:, q, :], es1b[:], AF.Relu,
                            bias=neg1[:], scale=es2m[:, jc : jc + 1],
                        )
                    else:
                        nc.vector.tensor_scalar(
                            out=u_t[:, q, :],
                            in0=es1b[:],
                            scalar1=es2m[:, jc : jc + 1],
                            scalar2=1.0,
                            op0=OP.mult,
                            op1=OP.max,
                        )
                n_t = ntp.tile([P, 4, MI], FP16, tag="n_t")
                nc.vector.tensor_tensor(
                    out=n_t[:, :sz, :], in0=u_t[:, :sz, :],
                    in1=adj_t[:, :sz, :], op=OP.mult,
                )
                for q in range(sz):
                    jc = off + q
                    gsl = gs[:, (jc % 8) * (F + 1) : (jc % 8) * (F + 1) + F + 1]
                    for it in range(NIT):
                        nc.tensor.matmul(
                            acc_slice(it),
                            n_t[:, q, it * P : (it + 1) * P],
                            gsl,
                            start=(jc == 0 and it % 2 == 0),
                            stop=(jc == NJ - 1),
                            skip_group_check=True,
                        )
                        if is_act:
                            nc.tensor.matmul(
                                acc_slice(it),
                                adj_t[:, q, it * P : (it + 1) * P],
                                gsl,
                                start=False,
                                stop=False,
                                skip_group_check=True,
                            )

            nunits = len(UNIT_SIZES)
            for un in range(nunits):
                phase_a(un)
                if un >= LAG:
                    phase_c()
            while pend:
                phase_c()

            # drain: vector-only copies into one batched tile, grouped DMAs
            out_r = out_d.rearrange("(c p) f -> p c f", p=P)
            res = outp.tile([P, NIT, F + 1], F32, tag="res")
            for g in range(4):
                for k in range(4):
                    it = 4 * g + k
                    nc.vector.tensor_copy(res[:, it, :], acc_slice(it))
                eng = nc.sync if g % 2 == 0 else nc.scalar
                eng.dma_start(
                    out=out_r[:, 4 * g : 4 * g + 4, :],
                    in_=res[:, 4 * g : 4 * g + 4, :],
                )

    nc.compile()
    return nc


_NC_CACHE = [None]


def _get_nc():
    if _NC_CACHE[0] is None:
        _NC_CACHE[0] = build_program()
    return _NC_CACHE[0]


def host_prepare(x, adj, W, a):
    h64 = x.astype(np.float64) @ W.astype(np.float64)
    s1 = h64 @ a[:F, 0].astype(np.float64)
    s2 = h64 @ a[F:, 0].astype(np.float64)
    es2a = np.exp(ALPHA * s2)
    es2m = np.exp((1.0 - ALPHA) * s2)
    g2 = np.empty((N, F + 1), np.float64)
    g2[:, :F] = h64
    g2[:, F] = 1.0
    g2 = g2.astype(np.float16)
    es1b = np.exp((1.0 - ALPHA) * s1).astype(np.float16)
    maskT = adj.T > 0
    mt_full = np.where(maskT, es2a[:, None], 0.0).astype(np.float16)

    in_maps = []
    for c in range(NCORES):
        si = c % 4
        hj = c // 4
        isl = slice(si * MI, (si + 1) * MI)
        jsl = slice(hj * MJ, (hj + 1) * MJ)
        g2h = np.ascontiguousarray(
            g2[jsl].reshape(NJ, P, F + 1).transpose(1, 0, 2).reshape(P, NJ * (F + 1))
        )
        es2mh = np.ascontiguousarray(es2m[jsl].reshape(NJ, P).T.astype(np.float32))
        in_maps.append(
            {
                "mt": np.ascontiguousarray(mt_full[jsl, isl]),
                "g2": g2h,
                "es1b": es1b[isl].reshape(1, MI),
                "es2m": es2mh,
            }
        )
    return in_maps


def kernel(x, adj, W, a, _trace=False):
    x = np.asarray(x)
    adj = np.asarray(adj)
    W = np.asarray(W)
    a = np.asarray(a)

    in_maps = host_prepare(x, adj, W, a)
    nc = _get_nc()
    res = bass_utils.run_bass_kernel_spmd(
        nc, in_maps, core_ids=list(range(NCORES)), trace=_trace
    )
    slabs = []
    for si in range(4):
        slabs.append(res.results[si]["out"] + res.results[si + 4]["out"])
    nd = np.concatenate(slabs, axis=0)
    hp = nd[:, :F] / nd[:, F : F + 1]
    out = np.where(hp > 0, hp, np.expm1(np.minimum(hp, 0.0))).astype(np.float32)
    if _trace:
        return out, res
    return out


# revision 6
# speedup vs baseline: 1.2087x; 1.1989x over previous
"""v17 Trainium2 Bass kernel for an attention-style graph convolution (GAT layer).

v7 = v3 (4x2 sharding, ACT relu-split offload) with progressive tiling:
the first j-chunks are processed as small units (1,1,2 chunks) so the
DMA->ts->tt->matmul pipeline fills fast, then steady-state runs on 4-chunk
units (low per-op overhead). Drain uses vector-only copies into one batched
tile and 4 grouped output DMAs on both rings.

Math (mask stream pre-scaled: mt[j,i] = m[i,j]*es2a_j, fp16):
    n[j,i] = mt * max(es1b_i*es2m_j, 1) = mt * (1 + relu(es1b_i*es2m_j - 1))
DVE-units: u = max(es1b*es2m_j, 1) (ts) ; n = u*mt (tt) ; 16 mm/chunk
ACT-units: w = relu(es1b*es2m_j - 1) (ACT engine) ; n1 = w*mt (tt)
           32 mm/chunk (n1.T@g2 plus mt.T@g2 -- the "+1" chain on the PE)
j-half partials summed across core pairs on the host, then divide + elu.
"""

import ml_dtypes
import numpy as np

import concourse.bacc as bacc
import concourse.bass as bass
import concourse.mybir as mybir
import concourse.tile as tile
from concourse import bass_utils

F32 = mybir.dt.float32
FP16 = mybir.dt.float16
AF = mybir.ActivationFunctionType
OP = mybir.AluOpType

N = 8192
K = 256
F = 128
ALPHA = 0.2
NCORES = 8
MI = 2048         # i-columns per core
MJ = 4096         # j-rows per core
P = 128
NJ = MJ // P      # 32 j-chunks
NIT = MI // P     # 16 i-blocks

UNIT_SIZES = [1, 1, 2, 4, 4, 4, 4, 4, 4, 2, 1, 1]  # tapered both ends (sum 32)
ACT_UNITS = frozenset({4, 6, 8})              # units built on the ACT engine
LAG = 2                                        # pipeline depth in units


def _broadcast_ap(row_ap, nparts):
    return bass.AP(
        tensor=row_ap.tensor,
        offset=row_ap.offset,
        ap=[[0, nparts]] + [list(d) for d in row_ap.ap],
    )


def build_program():
    nc = bacc.Bacc("TRN2", target_bir_lowering=False)

    mt_d = nc.dram_tensor("mt", (MJ, MI), FP16, kind="ExternalInput")
    g2_d = nc.dram_tensor("g2", (P, NJ * (F + 1)), FP16, kind="ExternalInput")
    es1b_d = nc.dram_tensor("es1b", (1, MI), FP16, kind="ExternalInput")
    es2m_d = nc.dram_tensor("es2m", (P, NJ), F32, kind="ExternalInput")
    out_d = nc.dram_tensor("out", (MI, F + 1), F32, kind="ExternalOutput")

    with tile.TileContext(nc) as tc:
        with (
            tc.tile_pool(name="consts", bufs=1) as consts,
            tc.tile_pool(name="adjp", bufs=4) as adjp,
            tc.tile_pool(name="up", bufs=2) as up,
            tc.tile_pool(name="ntp", bufs=3) as ntp,
            tc.tile_pool(name="gsp", bufs=2) as gsp,
            tc.tile_pool(name="outp", bufs=1) as outp,
            tc.tile_pool(name="ps_acc", bufs=1, space="PSUM") as ps_acc,
        ):
            es2m = consts.tile([P, NJ], F32, tag="es2m")
            es1b = consts.tile([P, MI], FP16, tag="es1b")
            neg1 = consts.tile([P, 1], F32, tag="neg1")
            nc.gpsimd.memset(neg1[:], -1.0)
            # es-vectors on sync; the first mask units go on scalar so the
            # latency-critical chunk-0 transfer is not queued behind es1b
            nc.sync.dma_start(out=es2m[:], in_=es2m_d[:, :])
            nc.sync.dma_start(out=es1b[:], in_=_broadcast_ap(es1b_d[:, :], P))

            accs = [
                ps_acc.tile([P, 512], F32, tag=f"acc{b}", name=f"acc{b}")
                for b in range(8)
            ]

            def acc_slice(it):
                return accs[it // 2][:, (it % 2) * 256 : (it % 2) * 256 + F + 1]

            mt_r = mt_d.rearrange("(c p) m -> p c m", p=P)
            unit_off = np.cumsum([0] + UNIT_SIZES).tolist()

            pend = []
            gs_slab = [None]

            def phase_a(un):
                off, sz = unit_off[un], UNIT_SIZES[un]
                if off % 8 == 0:
                    g8 = off // 8
                    gs = gsp.tile([P, 8 * (F + 1)], FP16, tag="gs")
                    nc.sync.dma_start(
                        out=gs[:],
                        in_=g2_d[:, g8 * 8 * (F + 1) : (g8 + 1) * 8 * (F + 1)],
                    )
                    gs_slab[0] = gs
                adj_t = adjp.tile([P, 4, MI], FP16, tag="adj")
                eng = nc.scalar if un % 2 == 0 else nc.sync
                eng.dma_start(
                    out=adj_t[:, :sz, :], in_=mt_r[:, off : off + sz, :]
                )
                pend.append((un, adj_t, gs_slab[0]))

            def phase_c():
                un, adj_t, gs = pend.pop(0)
                off, sz = unit_off[un], UNIT_SIZES[un]
                is_act = un in ACT_UNITS
                u_t = up.tile([P, 4, MI], FP16, tag="u_t")
                for q in range(sz):
                    jc = off + q
                    if is_act:
                        nc.scalar.activation(
                            u_t[:, q, :], es1b[:], AF.Relu,
                            bias=neg1[:], scale=es2m[:, jc : jc + 1],
                        )
                    else:
                        nc.vector.tensor_scalar(
                            out=u_t[:, q, :],
                            in0=es1b[:],
                            scalar1=es2m[:, jc : jc + 1],
                            scalar2=1.0,
                            op0=OP.mult,
                            op1=OP.max,
                        )
                n_t = ntp.tile([P, 4, MI], FP16, tag="n_t")
                nc.vector.tensor_tensor(
                    out=n_t[:, :sz, :], in0=u_t[:, :sz, :],
                    in1=adj_t[:, :sz, :], op=OP.mult,
                )
                for q in range(sz):
                    jc = off + q
                    gsl = gs[:, (jc % 8) * (F + 1) : (jc % 8) * (F + 1) + F + 1]
                    for it in range(NIT):
                        nc.tensor.matmul(
                            acc_slice(it),
                            n_t[:, q, it * P : (it + 1) * P],
                            gsl,
                            start=(jc == 0 and it % 2 == 0),
                            stop=(jc == NJ - 1),
                            skip_group_check=True,
                        )
                        if is_act:
                            nc.tensor.matmul(
                                acc_slice(it),
                                adj_t[:, q, it * P : (it + 1) * P],
                                gsl,
                                start=False,
                                stop=False,
                                skip_group_check=True,
                            )

            nunits = len(UNIT_SIZES)
            for un in range(nunits):
                phase_a(un)
                if un >= LAG:
                    phase_c()
            while pend:
                phase_c()

            # drain: vector-only copies into one batched tile, grouped DMAs
            out_r = out_d.rearrange("(c p) f -> p c f", p=P)
            res = outp.tile([P, NIT, F + 1], F32, tag="res")
            for g in range(4):
                for k in range(4):
                    it = 4 * g + k
                    if it % 2 == 0:
                        nc.vector.tensor_copy(res[:, it, :], acc_slice(it))
                    else:
                        nc.scalar.copy(res[:, it, :], acc_slice(it))
                eng = nc.sync if g % 2 == 0 else nc.scalar
                eng.dma_start(
                    out=out_r[:, 4 * g : 4 * g + 4, :],
                    in_=res[:, 4 * g : 4 * g + 4, :],
                )

    nc.compile()
    return nc


_NC_CACHE = [None]


def _get_nc():
    if _NC_CACHE[0] is None:
        _NC_CACHE[0] = build_program()
    return _NC_CACHE[0]


def host_prepare(x, adj, W, a):
    h64 = x.astype(np.float64) @ W.astype(np.float64)
    s1 = h64 @ a[:F, 0].astype(np.float64)
    s2 = h64 @ a[F:, 0].astype(np.float64)
    es2a = np.exp(ALPHA * s2)
    es2m = np.exp((1.0 - ALPHA) * s2)
    g2 = np.empty((N, F + 1), np.float64)
    g2[:, :F] = h64
    g2[:, F] = 1.0
    g2 = g2.astype(np.float16)
    es1b = np.exp((1.0 - ALPHA) * s1).astype(np.float16)
    maskT = adj.T > 0
    mt_full = np.where(maskT, es2a[:, None], 0.0).astype(np.float16)

    in_maps = []
    for c in range(NCORES):
        si = c % 4
        hj = c // 4
        isl = slice(si * MI, (si + 1) * MI)
        jsl = slice(hj * MJ, (hj + 1) * MJ)
        g2h = np.ascontiguousarray(
            g2[jsl].reshape(NJ, P, F + 1).transpose(1, 0, 2).reshape(P, NJ * (F + 1))
        )
        es2mh = np.ascontiguousarray(es2m[jsl].reshape(NJ, P).T.astype(np.float32))
        in_maps.append(
            {
                "mt": np.ascontiguousarray(mt_full[jsl, isl]),
                "g2": g2h,
                "es1b": es1b[isl].reshape(1, MI),
                "es2m": es2mh,
            }
        )
    return in_maps


def kernel(x, adj, W, a, _trace=False):
    x = np.asarray(x)
    adj = np.asarray(adj)
    W = np.asarray(W)
    a = np.asarray(a)

    in_maps = host_prepare(x, adj, W, a)
    nc = _get_nc()
    res = bass_utils.run_bass_kernel_spmd(
        nc, in_maps, core_ids=list(range(NCORES)), trace=_trace
    )
    slabs = []
    for si in range(4):
        slabs.append(res.results[si]["out"] + res.results[si + 4]["out"])
    nd = np.concatenate(slabs, axis=0)
    hp = nd[:, :F] / nd[:, F : F + 1]
    out = np.where(hp > 0, hp, np.expm1(np.minimum(hp, 0.0))).astype(np.float32)
    if _trace:
        return out, res
    return out


# revision 7
# speedup vs baseline: 1.3340x; 1.1037x over previous
"""v18 Trainium2 Bass kernel for an attention-style graph convolution (GAT).

Sorted-prefix skip: with i sorted by s1 (stratified mod-4 across the 4
i-slabs) and j sorted by s2 (stratified mod-2 across the 2 j-halves), the
region where w = relu(es1b_i*es2m_j - 1) == 0 (i.e. s1_i + s2_j <= 0) is a
block-aligned prefix of each j-chunk's i-axis. On those (chunk, i-block)
tiles n = mt exactly, so no ts/tt is needed -- just a mask-only matmul
whose mask streams as BINARY fp8 (exact), with es2a_j folded into a gQ
moving operand. This cuts the fp16 mask stream (the DMA spine) by the skip
fraction (~45%), replacing it with half-size fp8.

Per chunk c (128 j's), with k = k_list[c] skipped i-blocks (min over the 8
cores, so the single SPMD program is valid everywhere):
    blocks it <  k: acc[it] += m8[:, blk].T @ gQ_c     (fp8 mask, exact)
    blocks it >= k: u = max(es1b*es2m_c, 1) (ts) ; n = u*mt (tt)
                    acc[it] += n[:, blk'].T @ g2_c
Final: host sums j-half core pairs, un-permutes rows, divides by the
denominator column and applies elu.
"""

import ml_dtypes
import numpy as np

import concourse.bacc as bacc
import concourse.bass as bass
import concourse.mybir as mybir
import concourse.tile as tile
from concourse import bass_utils

F32 = mybir.dt.float32
FP16 = mybir.dt.float16
FP8 = mybir.dt.float8e4
OP = mybir.AluOpType

N = 8192
K = 256
F = 128
ALPHA = 0.2
NCORES = 8
MI = 2048         # i-columns per core (4 slabs, stratified)
MJ = 4096         # j-rows per core (2 halves, stratified)
P = 128
NJ = MJ // P      # 32 j-chunks
NIT = MI // P     # 16 i-blocks
LAG = 5           # pipeline depth in chunks
GW = F + 1        # g row width (128 features + denominator ones)


def _broadcast_ap(row_ap, nparts):
    return bass.AP(
        tensor=row_ap.tensor,
        offset=row_ap.offset,
        ap=[[0, nparts]] + [list(d) for d in row_ap.ap],
    )


def build_program(k_list):
    nc = bacc.Bacc("TRN2", target_bir_lowering=False)

    mt_d = nc.dram_tensor("mt", (MJ, MI), FP16, kind="ExternalInput")
    m8_d = nc.dram_tensor("m8", (MJ, MI), FP8, kind="ExternalInput")
    g2_d = nc.dram_tensor("g2", (P, NJ * GW), FP16, kind="ExternalInput")
    gq_d = nc.dram_tensor("gq", (P, NJ * GW), FP16, kind="ExternalInput")
    es1b_d = nc.dram_tensor("es1b", (1, MI), FP16, kind="ExternalInput")
    es2m_d = nc.dram_tensor("es2m", (P, NJ), F32, kind="ExternalInput")
    out_d = nc.dram_tensor("out", (MI, GW), F32, kind="ExternalOutput")

    with tile.TileContext(nc) as tc:
        with (
            tc.tile_pool(name="consts", bufs=1) as consts,
            tc.tile_pool(name="adjp", bufs=8) as adjp,
            tc.tile_pool(name="adj8p", bufs=6) as adj8p,
            tc.tile_pool(name="up", bufs=3) as up,
            tc.tile_pool(name="ntp", bufs=5) as ntp,
            tc.tile_pool(name="gsp", bufs=2) as gsp,
            tc.tile_pool(name="gqp", bufs=2) as gqp,
            tc.tile_pool(name="outp", bufs=1) as outp,
            tc.tile_pool(name="ps_acc", bufs=1, space="PSUM") as ps_acc,
        ):
            es2m = consts.tile([P, NJ], F32, tag="es2m")
            es1b = consts.tile([P, MI], FP16, tag="es1b")
            nc.sync.dma_start(out=es2m[:], in_=es2m_d[:, :])
            nc.sync.dma_start(out=es1b[:], in_=_broadcast_ap(es1b_d[:, :], P))

            accs = [
                ps_acc.tile([P, 512], F32, tag=f"acc{b}", name=f"acc{b}")
                for b in range(8)
            ]

            def acc_slice(it):
                return accs[it // 2][:, (it % 2) * 256 : (it % 2) * 256 + GW]

            mt_r = mt_d.rearrange("(c p) m -> p c m", p=P)
            m8_r = m8_d.rearrange("(c p) m -> p c m", p=P)

            pend = []
            slabs = [None, None]

            def phase_a(c):
                if c % 8 == 0:
                    g8 = c // 8
                    gs = gsp.tile([P, 8 * GW], FP16, tag="gs")
                    gq = gqp.tile([P, 8 * GW], FP16, tag="gq")
                    nc.sync.dma_start(
                        out=gs[:], in_=g2_d[:, g8 * 8 * GW : (g8 + 1) * 8 * GW]
                    )
                    nc.scalar.dma_start(
                        out=gq[:], in_=gq_d[:, g8 * 8 * GW : (g8 + 1) * 8 * GW]
                    )
                    slabs[0], slabs[1] = gs, gq
                k = k_list[c]
                w = MI - P * k
                adj_t = adj8_t = None
                if w > 0:
                    adj_t = adjp.tile([P, MI], FP16, tag="adj")
                    eng = nc.sync if c % 2 == 0 else nc.scalar
                    eng.dma_start(
                        out=adj_t[:, :w], in_=mt_r[:, c, P * k : MI]
                    )
                if k > 0:
                    adj8_t = adj8p.tile([P, MI], FP8, tag="adj8")
                    eng = nc.scalar if c % 2 == 0 else nc.sync
                    eng.dma_start(
                        out=adj8_t[:, : P * k], in_=m8_r[:, c, 0 : P * k]
                    )
                pend.append((c, adj_t, adj8_t, slabs[0], slabs[1]))

            def phase_c():
                c, adj_t, adj8_t, gs, gq = pend.pop(0)
                k = k_list[c]
                w = MI - P * k
                n_t = None
                if w > 0:
                    u_t = up.tile([P, MI], FP16, tag="u_t")
                    nc.vector.tensor_scalar(
                        out=u_t[:, :w],
                        in0=es1b[:, P * k : MI],
                        scalar1=es2m[:, c : c + 1],
                        scalar2=1.0,
                        op0=OP.mult,
                        op1=OP.max,
                    )
                    n_t = ntp.tile([P, MI], FP16, tag="n_t")
                    nc.vector.tensor_tensor(
                        out=n_t[:, :w], in0=u_t[:, :w], in1=adj_t[:, :w],
                        op=OP.mult,
                    )
                gsl = gs[:, (c % 8) * GW : (c % 8) * GW + GW]
                gql = gq[:, (c % 8) * GW : (c % 8) * GW + GW]
                for it in range(NIT):
                    if it < k:
                        nc.tensor.matmul(
                            acc_slice(it),
                            adj8_t[:, it * P : (it + 1) * P],
                            gql,
                            start=(c == 0 and it % 2 == 0),
                            stop=(c == NJ - 1),
                            skip_group_check=True,
                        )
                    else:
                        nc.tensor.matmul(
                            acc_slice(it),
                            n_t[:, it * P - P * k : (it + 1) * P - P * k],
                            gsl,
                            start=(c == 0 and it % 2 == 0),
                            stop=(c == NJ - 1),
                            skip_group_check=True,
                        )

            for c in range(NJ):
                phase_a(c)
                if c >= LAG:
                    phase_c()
            while pend:
                phase_c()

            out_r = out_d.rearrange("(c p) f -> p c f", p=P)
            res = outp.tile([P, NIT, GW], F32, tag="res")
            for g in range(4):
                for kk in range(4):
                    it = 4 * g + kk
                    if it % 2 == 0:
                        nc.vector.tensor_copy(res[:, it, :], acc_slice(it))
                    else:
                        nc.scalar.copy(res[:, it, :], acc_slice(it))
                eng = nc.sync if g % 2 == 0 else nc.scalar
                eng.dma_start(
                    out=out_r[:, 4 * g : 4 * g + 4, :],
                    in_=res[:, 4 * g : 4 * g + 4, :],
                )

    nc.compile()
    return nc


def host_prepare(x, adj, W, a):
    h64 = x.astype(np.float64) @ W.astype(np.float64)
    s1 = h64 @ a[:F, 0].astype(np.float64)
    s2 = h64 @ a[F:, 0].astype(np.float64)
    es2a = np.exp(ALPHA * s2)
    es2m = np.exp((1.0 - ALPHA) * s2)
    g2 = np.empty((N, GW), np.float64)
    g2[:, :F] = h64
    g2[:, F] = 1.0
    gq64 = g2 * es2a[:, None]
    g2 = g2.astype(np.float16)
    gq = gq64.astype(np.float16)
    es1b16 = np.exp((1.0 - ALPHA) * s1).astype(np.float16)

    # stratified sorted sharding: i by s1 (mod 4 slabs), j by s2 (mod 2 halves)
    isort = np.argsort(s1, kind="stable")
    ilists = [isort[sl::4] for sl in range(4)]          # each ascending in s1
    jsort = np.argsort(s2, kind="stable")
    jlists = [jsort[h::2] for h in range(2)]            # each ascending in s2

    maskT = adj.T > 0

    # skip table: tile (chunk c, block b) is skippable iff
    # max(es1b16[block]) * max(es2m[chunk]) <= 1  (exact device comparison)
    k_per_core = []
    for h in range(2):
        es2m_h = es2m[jlists[h]].astype(np.float32)
        cmax = es2m_h.reshape(NJ, P).max(axis=1)        # per-chunk max
        for sl in range(4):
            bmax = (
                es1b16[ilists[sl]].astype(np.float32).reshape(NIT, P).max(axis=1)
            )
            # blocks sorted ascending -> bmax ascending; prefix property holds
            k_c = (bmax[None, :] * cmax[:, None] <= 1.0).sum(axis=1)
            k_per_core.append(k_c)
    k_list = np.minimum.reduce(k_per_core).astype(int).tolist()

    in_maps = []
    for c in range(NCORES):
        sl = c % 4
        h = c // 4
        il, jl = ilists[sl], jlists[h]
        mT = maskT[np.ix_(jl, il)]
        es2a_j = es2a[jl]
        mt = np.where(mT, es2a_j[:, None], 0.0).astype(np.float16)
        m8 = mT.astype(ml_dtypes.float8_e4m3)
        g2h = np.ascontiguousarray(
            g2[jl].reshape(NJ, P, GW).transpose(1, 0, 2).reshape(P, NJ * GW)
        )
        gqh = np.ascontiguousarray(
            gq[jl].reshape(NJ, P, GW).transpose(1, 0, 2).reshape(P, NJ * GW)
        )
        es2mh = np.ascontiguousarray(
            es2m[jl].reshape(NJ, P).T.astype(np.float32)
        )
        in_maps.append(
            {
                "mt": np.ascontiguousarray(mt),
                "m8": np.ascontiguousarray(m8),
                "g2": g2h,
                "gq": gqh,
                "es1b": es1b16[il].reshape(1, MI),
                "es2m": es2mh,
            }
        )
    return in_maps, k_list, ilists


_NC_CACHE = {}


def kernel(x, adj, W, a, _trace=False):
    x = np.asarray(x)
    adj = np.asarray(adj)
    W = np.asarray(W)
    a = np.asarray(a)

    in_maps, k_list, ilists = host_prepare(x, adj, W, a)
    key = tuple(k_list)
    if key not in _NC_CACHE:
        _NC_CACHE.clear()
        _NC_CACHE[key] = build_program(k_list)
    nc = _NC_CACHE[key]
    res = bass_utils.run_bass_kernel_spmd(
        nc, in_maps, core_ids=list(range(NCORES)), trace=_trace
    )
    nd = np.empty((N, GW), np.float32)
    for sl in range(4):
        nd[ilists[sl]] = res.results[sl]["out"] + res.results[sl + 4]["out"]
    hp = nd[:, :F] / nd[:, F : F + 1]
    out = np.where(hp > 0, hp, np.expm1(np.minimum(hp, 0.0))).astype(np.float32)
    if _trace:
        return out, res
    return out
